# revision 12
# baseline (speedup 1.0000x reference)
"""Trainium2 Bass kernel for cross-attention (B=8, NQ=1024, NKV=512, H=16, D=64).

Sharding: pure data-parallel — one batch element per NeuronCore (8 cores).

Per-core dataflow (all matmuls via TensorE; matmul operands in `mm_dt`,
float32r by default, which streams 4x faster than float32 on trn2):
  kT[o,j]  = sum_cc WkT[cc,o] * ctxT[cc,j] + bk[o]          (K projection, transposed)
  v[j,o]   = sum_cc ctxT[cc,j] * WvT[cc,o] + bv[o]          (V projection, natural;
             stored per-head with an extra all-ones column -> v_aug[j, h, 0:65])
  qT[o,i]  = sum_c  WqT[c,o] * xT[c,i] + bq[o]              (Q projection, transposed;
             softmax scale folded into WqT/bq on host)
  sT[j,i]  = sum_d kT[hd+d,j] * qT[hd+d,i]                  (scores, transposed, per head)
  p[j,i]   = exp(sT[j,i])                                   (unnormalized probs, ACT engine)
  pv[0:65,i] = sum_j v_aug[j,h,:] * p[j,i]                  (rows 0..63 = attn out^T,
                                                             row 64 = softmax denominator)
  r[i]     = 1/pv[64,i]; bcast via K=1 matmul (ones x r) -> [64,i] in PSUM -> SBUF
  oaT[o,i] = pv[0:64,i] * bcast                             (normalized attn out, transposed)
  out[i,m] = sum_o oaT[o,i] * WoT[o,m] + bo[m]              (output projection)

No max-subtraction in softmax: q is pre-scaled by 1/sqrt(D), scores are ~N(0,1),
|s| < ~15 with overwhelming margin, exp() is safe in fp32.
"""

import os
import sys

import numpy as np

for _p in ("/opt/trn_rl_repo",):
    if os.path.isdir(_p) and _p not in sys.path:
        sys.path.insert(0, _p)

import concourse.bass as bass
import concourse.tile as tile
from concourse import bacc, mybir
from concourse.bass_utils import run_bass_kernel_spmd

F32 = mybir.dt.float32

B = 8
NQ = 1024
NKV = 512
IN_DIM = 1024
CTX_DIM = 768
H = 16
D = 64
SCALE = 1.0 / 8.0  # 1/sqrt(64)

P = 128
NB = 512  # matmul moving free-dim block

N_CORES = 8


def _emit(tc, aps, mm_dt):
    nc = tc.nc
    MDT = mm_dt  # dtype of every tile that feeds the PE array

    xT, ctxT, wqT, wkT, wvT, woT, bqd, bkd, bvd, bod, out = aps

    from contextlib import ExitStack

    ctx = tc._ctx = ExitStack()

    # pools alive for the whole kernel
    singles = ctx.enter_context(tc.tile_pool(name="singles", bufs=1))
    kt_pool = ctx.enter_context(tc.tile_pool(name="kt", bufs=8))
    vaug_pool = ctx.enter_context(tc.tile_pool(name="vaug", bufs=4))
    qt_pool = ctx.enter_context(tc.tile_pool(name="qt", bufs=8))
    oat_pool = ctx.enter_context(tc.tile_pool(name="oat", bufs=8))

    proj_ps = ctx.enter_context(tc.tile_pool(name="proj_ps", bufs=2, space="PSUM"))
    st_ps = ctx.enter_context(tc.tile_pool(name="st_ps", bufs=3, space="PSUM"))
    pv_ps = ctx.enter_context(tc.tile_pool(name="pv_ps", bufs=2, space="PSUM"))
    bc_ps = ctx.enter_context(tc.tile_pool(name="bc_ps", bufs=1, space="PSUM"))

    Exp = mybir.ActivationFunctionType.Exp

    # ---- constants / biases ----
    # per-partition bias tiles: bias[p, t] = b[t*128 + p]
    bq_sb = singles.tile([P, IN_DIM // P], F32, tag="bq")
    nc.sync.dma_start(out=bq_sb, in_=bqd.rearrange("(t p) -> p t", p=P))
    bk_sb = singles.tile([P, IN_DIM // P], F32, tag="bk")
    nc.sync.dma_start(out=bk_sb, in_=bkd.rearrange("(t p) -> p t", p=P))
    # free-dim (broadcast along partitions) bias tiles
    bv_sb = singles.tile([P, IN_DIM], F32, tag="bv")
    nc.sync.dma_start(
        out=bv_sb,
        in_=bass.AP(tensor=bvd.tensor, offset=bvd.offset, ap=[[0, P]] + list(bvd.ap)),
    )
    bo_sb = singles.tile([P, IN_DIM], F32, tag="bo")
    nc.sync.dma_start(
        out=bo_sb,
        in_=bass.AP(tensor=bod.tensor, offset=bod.offset, ap=[[0, P]] + list(bod.ap)),
    )
    ones65 = singles.tile([65, D], MDT, tag="ones65")
    onesc = singles.tile([P, H, 1], MDT, tag="onesc")
    if MDT == F32:
        nc.vector.memset(ones65[:], 1.0)
        nc.vector.memset(onesc[:], 1.0)
    else:
        ones65_f = singles.tile([65, D], F32, tag="ones65_f")
        nc.vector.memset(ones65_f[:], 1.0)
        nc.vector.tensor_copy(ones65[:], ones65_f[:])
        onesc_f = singles.tile([P, H, 1], F32, tag="onesc_f")
        nc.vector.memset(onesc_f[:], 1.0)
        nc.vector.tensor_copy(onesc[:], onesc_f[:])

    CCT = CTX_DIM // P  # 6 context-dim tiles
    OT = IN_DIM // P  # 8 o-tiles
    JT = NKV // P  # 4 key tiles
    IBLK = NQ // NB  # 2 query blocks
    MBLK = IN_DIM // NB  # 2 output blocks

    # ================= phase 1: K/V projections =================
    kt = []
    v_aug = []
    with ExitStack() as ph1:
        ctx_pool = ph1.enter_context(tc.tile_pool(name="ctxp", bufs=6))
        wk_pool = ph1.enter_context(tc.tile_pool(name="wk", bufs=6))
        wv_pool = ph1.enter_context(tc.tile_pool(name="wv", bufs=6))

        ctx_t = []
        for cc in range(CCT):
            t = ctx_pool.tile([P, NKV], MDT, tag="ctx", name=f"ctx{cc}")
            nc.sync.dma_start(out=t, in_=ctxT[cc * P : (cc + 1) * P, :])
            ctx_t.append(t)

        # ---- K projection: kT[o, j] ----
        wk_t = []
        for cc in range(CCT):
            t = wk_pool.tile([P, IN_DIM], MDT, tag="wk", name=f"wk{cc}")
            nc.sync.dma_start(out=t, in_=wkT[cc * P : (cc + 1) * P, :])
            wk_t.append(t)
        for ot in range(OT):
            ps = proj_ps.tile([P, NKV], F32, tag="proj", name=f"kps{ot}")
            for cc in range(CCT):
                nc.tensor.matmul(
                    out=ps,
                    lhsT=wk_t[cc][:, ot * P : (ot + 1) * P],
                    rhs=ctx_t[cc][:],
                    start=(cc == 0),
                    stop=(cc == CCT - 1),
                )
            t = kt_pool.tile([P, NKV], MDT, tag="kt", name=f"kt{ot}")
            nc.vector.tensor_scalar_add(t, ps, bk_sb[:, ot : ot + 1])
            kt.append(t)

        # ---- V projection into v_aug[j, h, 0:65] (col 64 = ones) ----
        wv_t = []
        for cc in range(CCT):
            t = wv_pool.tile([P, IN_DIM], MDT, tag="wv", name=f"wv{cc}")
            nc.sync.dma_start(out=t, in_=wvT[cc * P : (cc + 1) * P, :])
            wv_t.append(t)
        for jt in range(JT):
            va = vaug_pool.tile([P, H, 65], MDT, tag="vaug", name=f"vaug{jt}")
            if MDT == F32:
                nc.vector.memset(va[:, :, 64:65], 1.0)
            else:
                nc.vector.tensor_copy(va[:, :, 64:65], onesc[:])
            v_aug.append(va)
        for jt in range(JT):
            for oblk in range(MBLK):
                ps = proj_ps.tile([P, NB], F32, tag="proj", name=f"vps{jt}_{oblk}")
                for cc in range(CCT):
                    nc.tensor.matmul(
                        out=ps,
                        lhsT=ctx_t[cc][:, jt * P : (jt + 1) * P],
                        rhs=wv_t[cc][:, oblk * NB : (oblk + 1) * NB],
                        start=(cc == 0),
                        stop=(cc == CCT - 1),
                    )
                hpb = NB // D  # 8 heads per block
                nc.vector.tensor_add(
                    v_aug[jt][:, oblk * hpb : (oblk + 1) * hpb, 0:64],
                    ps.rearrange("p (h d) -> p h d", d=D),
                    bv_sb[:, oblk * NB : (oblk + 1) * NB].rearrange(
                        "p (h d) -> p h d", d=D
                    ),
                )

    # ================= phase 2: Q projection =================
    qt = []
    CT = IN_DIM // P  # 8 c-tiles
    with ExitStack() as ph2:
        xt_pool = ph2.enter_context(tc.tile_pool(name="xt", bufs=2 * CT))
        wq_pool = ph2.enter_context(tc.tile_pool(name="wq", bufs=18))
        xt = {}
        for c in range(CT):
            for ib in range(IBLK):
                t = xt_pool.tile([P, NB], MDT, tag="xt", name=f"xt{c}_{ib}")
                nc.sync.dma_start(
                    out=t, in_=xT[c * P : (c + 1) * P, ib * NB : (ib + 1) * NB]
                )
                xt[(c, ib)] = t
        for ot in range(OT):
            qtile = qt_pool.tile([P, NQ], MDT, tag="qt", name=f"qt{ot}")
            wq_t = []
            for c in range(CT):
                t = wq_pool.tile([P, P], MDT, tag="wq", name=f"wq{ot}_{c}")
                nc.sync.dma_start(
                    out=t, in_=wqT[c * P : (c + 1) * P, ot * P : (ot + 1) * P]
                )
                wq_t.append(t)
            for ib in range(IBLK):
                ps = proj_ps.tile([P, NB], F32, tag="proj", name=f"qps{ot}_{ib}")
                for c in range(CT):
                    nc.tensor.matmul(
                        out=ps,
                        lhsT=wq_t[c][:],
                        rhs=xt[(c, ib)][:],
                        start=(c == 0),
                        stop=(c == CT - 1),
                    )
                nc.vector.tensor_scalar_add(
                    qtile[:, ib * NB : (ib + 1) * NB], ps, bq_sb[:, ot : ot + 1]
                )
            qt.append(qtile)

    # ================= phase 3: attention (+ prefetch Wo) =================
    wo_pool = ctx.enter_context(tc.tile_pool(name="wo", bufs=8))
    wo_t = []
    for ot in range(OT):
        t = wo_pool.tile([P, IN_DIM], MDT, tag="wo", name=f"wo{ot}")
        nc.sync.dma_start(out=t, in_=woT[ot * P : (ot + 1) * P, :])
        wo_t.append(t)

    oat = [oat_pool.tile([P, NQ], MDT, tag="oat", name=f"oat{i}") for i in range(OT)]
    with ExitStack() as ph3:
        probs_pool = ph3.enter_context(tc.tile_pool(name="probs", bufs=10))
        stage_pool = ph3.enter_context(tc.tile_pool(name="stage", bufs=3))
        scr_pool = ph3.enter_context(tc.tile_pool(name="scr", bufs=6))
        for ot in range(OT):
            for ib in range(IBLK):
                for hh in range(2):
                    h = 2 * ot + hh
                    lo, hi = hh * D, (hh + 1) * D
                    probs = []
                    for jt in range(JT):
                        ps = st_ps.tile([P, NB], F32, tag="st", name=f"st{h}_{ib}_{jt}")
                        nc.tensor.matmul(
                            out=ps,
                            lhsT=kt[ot][lo:hi, jt * P : (jt + 1) * P],
                            rhs=qt[ot][lo:hi, ib * NB : (ib + 1) * NB],
                            start=True,
                            stop=True,
                        )
                        pt = probs_pool.tile(
                            [P, NB], MDT, tag="probs", name=f"pr{h}_{ib}_{jt}"
                        )
                        nc.scalar.activation(pt, ps, Exp)
                        probs.append(pt)
                    pv = pv_ps.tile([65, NB], F32, tag="pv", name=f"pv{h}_{ib}")
                    for jt in range(JT):
                        nc.tensor.matmul(
                            out=pv,
                            lhsT=v_aug[jt][:, h, :],
                            rhs=probs[jt][:],
                            start=(jt == 0),
                            stop=(jt == JT - 1),
                        )
                    st = stage_pool.tile([65, NB], MDT, tag="stage", name=f"rc{h}_{ib}")
                    with nc.allow_low_precision(reason="fp32r matmul operand"):
                        nc.vector.reciprocal(st[64:65, :], pv[64:65, :])
                    bc = bc_ps.tile([D, NB], F32, tag="bc", name=f"bc{h}_{ib}")
                    nc.tensor.matmul(
                        out=bc,
                        lhsT=ones65[64:65, :],
                        rhs=st[64:65, :],
                        start=True,
                        stop=True,
                    )
                    bcs = scr_pool.tile([D, NB], F32, tag="bcs", name=f"bs{h}_{ib}")
                    nc.vector.tensor_copy(bcs, bc[:])
                    if hh == 0:
                        nc.vector.tensor_mul(
                            oat[ot][0:64, ib * NB : (ib + 1) * NB], pv[0:64, :], bcs[:]
                        )
                    else:
                        sc = scr_pool.tile([D, NB], MDT, tag="scr", name=f"sc{h}_{ib}")
                        nc.vector.tensor_mul(sc, pv[0:64, :], bcs[:])
                        nc.sync.dma_start(
                            out=oat[ot][64:128, ib * NB : (ib + 1) * NB], in_=sc
                        )

    # ================= phase 4: output projection: out[i, m] =================
    outsb_pool = ctx.enter_context(tc.tile_pool(name="outsb", bufs=3))
    for isub in range(NQ // P):
        for mb in range(MBLK):
            ps = proj_ps.tile([P, NB], F32, tag="proj", name=f"ops{isub}_{mb}")
            for ot in range(OT):
                nc.tensor.matmul(
                    out=ps,
                    lhsT=oat[ot][:, isub * P : (isub + 1) * P],
                    rhs=wo_t[ot][:, mb * NB : (mb + 1) * NB],
                    start=(ot == 0),
                    stop=(ot == OT - 1),
                )
            ob = outsb_pool.tile([P, NB], F32, tag="outsb", name=f"ob{isub}_{mb}")
            nc.vector.tensor_add(ob, ps, bo_sb[:, mb * NB : (mb + 1) * NB])
            nc.sync.dma_start(
                out=out[isub * P : (isub + 1) * P, mb * NB : (mb + 1) * NB], in_=ob
            )

    ctx.close()


def build_nc(mm_dt=mybir.dt.float32r):
    nc = bacc.Bacc(
        "TRN2",
        target_bir_lowering=False,
        debug=False,
        num_devices=N_CORES,
    )
    aps = (
        nc.dram_tensor("xT", [IN_DIM, NQ], mm_dt, kind="ExternalInput").ap(),
        nc.dram_tensor("ctxT", [CTX_DIM, NKV], mm_dt, kind="ExternalInput").ap(),
        nc.dram_tensor("wqT", [IN_DIM, IN_DIM], mm_dt, kind="ExternalInput").ap(),
        nc.dram_tensor("wkT", [CTX_DIM, IN_DIM], mm_dt, kind="ExternalInput").ap(),
        nc.dram_tensor("wvT", [CTX_DIM, IN_DIM], mm_dt, kind="ExternalInput").ap(),
        nc.dram_tensor("woT", [IN_DIM, IN_DIM], mm_dt, kind="ExternalInput").ap(),
        nc.dram_tensor("bq", [IN_DIM], F32, kind="ExternalInput").ap(),
        nc.dram_tensor("bk", [IN_DIM], F32, kind="ExternalInput").ap(),
        nc.dram_tensor("bv", [IN_DIM], F32, kind="ExternalInput").ap(),
        nc.dram_tensor("bo", [IN_DIM], F32, kind="ExternalInput").ap(),
        nc.dram_tensor("out", [NQ, IN_DIM], F32, kind="ExternalOutput").ap(),
    )
    with tile.TileContext(nc) as tc:
        _emit(tc, aps, mm_dt)
    nc.compile()
    return nc


_NC_CACHE = {}


def get_nc(mm_dt=mybir.dt.float32r):
    key = str(mm_dt)
    if key not in _NC_CACHE:
        _NC_CACHE[key] = build_nc(mm_dt)
    return _NC_CACHE[key]


def make_in_maps(x, context, Wq, bq, Wk, bk, Wv, bv, Wo, bo):
    f = lambda a: np.asarray(a, dtype=np.float32)
    WqT = np.ascontiguousarray(f(Wq).T * SCALE)
    WkT = np.ascontiguousarray(f(Wk).T)
    WvT = np.ascontiguousarray(f(Wv).T)
    WoT = np.ascontiguousarray(f(Wo).T)
    bq_s = f(bq) * SCALE
    bk, bv, bo = f(bk), f(bv), f(bo)
    x, context = f(x), f(context)
    in_maps = []
    for b in range(B):
        in_maps.append(
            {
                "xT": np.ascontiguousarray(x[b].T),
                "ctxT": np.ascontiguousarray(context[b].T),
                "wqT": WqT,
                "wkT": WkT,
                "wvT": WvT,
                "woT": WoT,
                "bq": bq_s,
                "bk": bk,
                "bv": bv,
                "bo": bo,
            }
        )
    return in_maps


def run(in_maps, mm_dt=mybir.dt.float32r, trace=False, **kw):
    nc = get_nc(mm_dt)
    return run_bass_kernel_spmd(nc, in_maps, list(range(N_CORES)), trace=trace, **kw)


def kernel(x, context, Wq, bq, Wk, bk, Wv, bv, Wo, bo):
    in_maps = make_in_maps(x, context, Wq, bq, Wk, bk, Wv, bv, Wo, bo)
    res = run(in_maps).results
    return np.stack([res[b]["out"] for b in range(B)], axis=0)


# revision 21
# speedup vs baseline: 1.3652x; 1.3652x over previous
"""Trainium2 Bass kernel for cross-attention (B=8, NQ=1024, NKV=512, H=16, D=64).

Sharding: pure data-parallel — one batch element per NeuronCore (8 cores).

Per-core dataflow (all matmuls via TensorE; matmul operands in `mm_dt`,
float32r by default, which streams 4x faster than float32 on trn2):
  kT[o,j]  = sum_cc WkT[cc,o] * ctxT[cc,j] + bk[o]          (K projection, transposed)
  v[j,o]   = sum_cc ctxT[cc,j] * WvT[cc,o] + bv[o]          (V projection, natural;
             stored per-head with an extra all-ones column -> v_aug[j, h, 0:65])
  qT[o,i]  = sum_c  WqT[c,o] * xT[c,i] + bq[o]              (Q projection, transposed;
             softmax scale folded into WqT/bq on host)
  sT[j,i]  = sum_d kT[hd+d,j] * qT[hd+d,i]                  (scores, transposed, per head)
  p[j,i]   = exp(sT[j,i])                                   (unnormalized probs, ACT engine)
  pv[0:65,i] = sum_j v_aug[j,h,:] * p[j,i]                  (rows 0..63 = attn out^T,
                                                             row 64 = softmax denominator)
  r[i]     = 1/pv[64,i]; bcast via K=1 matmul (ones x r) -> [64,i] in PSUM -> SBUF
  oaT[o,i] = pv[0:64,i] * bcast                             (normalized attn out, transposed)
  out[i,m] = sum_o oaT[o,i] * WoT[o,m] + bo[m]              (output projection)

No max-subtraction in softmax: q is pre-scaled by 1/sqrt(D), scores are ~N(0,1),
|s| < ~15 with overwhelming margin, exp() is safe in fp32.
"""

import os
import sys

import numpy as np

for _p in ("/opt/trn_rl_repo",):
    if os.path.isdir(_p) and _p not in sys.path:
        sys.path.insert(0, _p)

import concourse.bass as bass
import concourse.tile as tile
from concourse import bacc, mybir
from concourse.bass_utils import run_bass_kernel_spmd

F32 = mybir.dt.float32

B = 8
NQ = 1024
NKV = 512
IN_DIM = 1024
CTX_DIM = 768
H = 16
D = 64
SCALE = 1.0 / 8.0  # 1/sqrt(64)

P = 128
NB = 512  # matmul moving free-dim block

N_CORES = 8


def _emit(tc, aps, mm_dt):
    nc = tc.nc
    MDT = mm_dt  # dtype of every tile that feeds the PE array

    xT, ctxT, wqT, wkT, wvT, woT, bqd, bkd, bvd, bod, out = aps

    from contextlib import ExitStack

    ctx = tc._ctx = ExitStack()

    # pools alive for the whole kernel
    singles = ctx.enter_context(tc.tile_pool(name="singles", bufs=1))
    kt_pool = ctx.enter_context(tc.tile_pool(name="kt", bufs=8))
    vaug_pool = ctx.enter_context(tc.tile_pool(name="vaug", bufs=4))
    qt_pool = ctx.enter_context(tc.tile_pool(name="qt", bufs=8))
    oat_pool = ctx.enter_context(tc.tile_pool(name="oat", bufs=8))

    proj_ps = ctx.enter_context(tc.tile_pool(name="proj_ps", bufs=2, space="PSUM"))
    st_ps = ctx.enter_context(tc.tile_pool(name="st_ps", bufs=4, space="PSUM"))
    pv_ps = ctx.enter_context(tc.tile_pool(name="pv_ps", bufs=2, space="PSUM"))

    Exp = mybir.ActivationFunctionType.Exp

    # ---- constants / biases ----
    # per-partition bias tiles: bias[p, t] = b[t*128 + p]
    bq_sb = singles.tile([P, IN_DIM // P], F32, tag="bq")
    nc.sync.dma_start(out=bq_sb, in_=bqd.rearrange("(t p) -> p t", p=P))
    bk_sb = singles.tile([P, IN_DIM // P], F32, tag="bk")
    nc.sync.dma_start(out=bk_sb, in_=bkd.rearrange("(t p) -> p t", p=P))
    # free-dim (broadcast along partitions) bias tiles
    bv_sb = singles.tile([P, IN_DIM], F32, tag="bv")
    nc.sync.dma_start(
        out=bv_sb,
        in_=bass.AP(tensor=bvd.tensor, offset=bvd.offset, ap=[[0, P]] + list(bvd.ap)),
    )
    bo_sb = singles.tile([P, IN_DIM], F32, tag="bo")
    nc.sync.dma_start(
        out=bo_sb,
        in_=bass.AP(tensor=bod.tensor, offset=bod.offset, ap=[[0, P]] + list(bod.ap)),
    )
    onesc = singles.tile([P, H, 1], MDT, tag="onesc")
    if MDT == F32:
        nc.vector.memset(onesc[:], 1.0)
    else:
        onesc_f = singles.tile([P, H, 1], F32, tag="onesc_f")
        nc.vector.memset(onesc_f[:], 1.0)
        nc.vector.tensor_copy(onesc[:], onesc_f[:])
    # DRAM bounce buffer for softmax reciprocal rows (partition-broadcast source)
    recd = nc.dram_tensor("recd", [NQ // NB, H, NB], F32).ap()

    CCT = CTX_DIM // P  # 6 context-dim tiles
    OT = IN_DIM // P  # 8 o-tiles
    JT = NKV // P  # 4 key tiles
    IBLK = NQ // NB  # 2 query blocks
    MBLK = IN_DIM // NB  # 2 output blocks

    # ================= phase 1: K/V projections =================
    kt = []
    v_aug = []
    with ExitStack() as ph1:
        ctx_pool = ph1.enter_context(tc.tile_pool(name="ctxp", bufs=6))
        wk_pool = ph1.enter_context(tc.tile_pool(name="wk", bufs=6))
        wv_pool = ph1.enter_context(tc.tile_pool(name="wv", bufs=6))

        ctx_t = []
        for cc in range(CCT):
            t = ctx_pool.tile([P, NKV], MDT, tag="ctx", name=f"ctx{cc}")
            nc.sync.dma_start(out=t, in_=ctxT[cc * P : (cc + 1) * P, :])
            ctx_t.append(t)

        # ---- K projection: kT[o, j] ----
        wk_t = []
        for cc in range(CCT):
            t = wk_pool.tile([P, IN_DIM], MDT, tag="wk", name=f"wk{cc}")
            nc.sync.dma_start(out=t, in_=wkT[cc * P : (cc + 1) * P, :])
            wk_t.append(t)
        for ot in range(OT):
            ps = proj_ps.tile([P, NKV], F32, tag="proj", name=f"kps{ot}")
            for cc in range(CCT):
                nc.tensor.matmul(
                    out=ps,
                    lhsT=wk_t[cc][:, ot * P : (ot + 1) * P],
                    rhs=ctx_t[cc][:],
                    start=(cc == 0),
                    stop=(cc == CCT - 1),
                )
            t = kt_pool.tile([P, NKV], MDT, tag="kt", name=f"kt{ot}")
            nc.vector.tensor_scalar_add(t, ps, bk_sb[:, ot : ot + 1])
            kt.append(t)

        # ---- V projection into v_aug[j, h, 0:65] (col 64 = ones) ----
        wv_t = []
        for cc in range(CCT):
            t = wv_pool.tile([P, IN_DIM], MDT, tag="wv", name=f"wv{cc}")
            nc.sync.dma_start(out=t, in_=wvT[cc * P : (cc + 1) * P, :])
            wv_t.append(t)
        for jt in range(JT):
            va = vaug_pool.tile([P, H, 65], MDT, tag="vaug", name=f"vaug{jt}")
            if MDT == F32:
                nc.vector.memset(va[:, :, 64:65], 1.0)
            else:
                nc.vector.tensor_copy(va[:, :, 64:65], onesc[:])
            v_aug.append(va)
        for jt in range(JT):
            for oblk in range(MBLK):
                ps = proj_ps.tile([P, NB], F32, tag="proj", name=f"vps{jt}_{oblk}")
                for cc in range(CCT):
                    nc.tensor.matmul(
                        out=ps,
                        lhsT=ctx_t[cc][:, jt * P : (jt + 1) * P],
                        rhs=wv_t[cc][:, oblk * NB : (oblk + 1) * NB],
                        start=(cc == 0),
                        stop=(cc == CCT - 1),
                    )
                hpb = NB // D  # 8 heads per block
                nc.vector.tensor_add(
                    v_aug[jt][:, oblk * hpb : (oblk + 1) * hpb, 0:64],
                    ps.rearrange("p (h d) -> p h d", d=D),
                    bv_sb[:, oblk * NB : (oblk + 1) * NB].rearrange(
                        "p (h d) -> p h d", d=D
                    ),
                )

    # ================= phase 2: Q projection =================
    qt = []
    CT = IN_DIM // P  # 8 c-tiles
    with ExitStack() as ph2:
        xt_pool = ph2.enter_context(tc.tile_pool(name="xt", bufs=2 * CT))
        wq_pool = ph2.enter_context(tc.tile_pool(name="wq", bufs=18))
        xt = {}
        for c in range(CT):
            for ib in range(IBLK):
                t = xt_pool.tile([P, NB], MDT, tag="xt", name=f"xt{c}_{ib}")
                nc.sync.dma_start(
                    out=t, in_=xT[c * P : (c + 1) * P, ib * NB : (ib + 1) * NB]
                )
                xt[(c, ib)] = t
        for ot in range(OT):
            qtile = qt_pool.tile([P, NQ], MDT, tag="qt", name=f"qt{ot}")
            wq_t = []
            for c in range(CT):
                t = wq_pool.tile([P, P], MDT, tag="wq", name=f"wq{ot}_{c}")
                nc.sync.dma_start(
                    out=t, in_=wqT[c * P : (c + 1) * P, ot * P : (ot + 1) * P]
                )
                wq_t.append(t)
            for ib in range(IBLK):
                ps = proj_ps.tile([P, NB], F32, tag="proj", name=f"qps{ot}_{ib}")
                for c in range(CT):
                    nc.tensor.matmul(
                        out=ps,
                        lhsT=wq_t[c][:],
                        rhs=xt[(c, ib)][:],
                        start=(c == 0),
                        stop=(c == CT - 1),
                    )
                nc.vector.tensor_scalar_add(
                    qtile[:, ib * NB : (ib + 1) * NB], ps, bq_sb[:, ot : ot + 1]
                )
            qt.append(qtile)

    # ================= phase 3: attention (+ prefetch Wo) =================
    wo_pool = ctx.enter_context(tc.tile_pool(name="wo", bufs=8))
    wo_t = []
    for ot in range(OT):
        t = wo_pool.tile([P, IN_DIM], MDT, tag="wo", name=f"wo{ot}")
        nc.sync.dma_start(out=t, in_=woT[ot * P : (ot + 1) * P, :])
        wo_t.append(t)

    oat = [oat_pool.tile([P, NQ], MDT, tag="oat", name=f"oat{i}") for i in range(OT)]
    with ExitStack() as ph3:
        probs_pool = ph3.enter_context(tc.tile_pool(name="probs", bufs=6))
        oatu_pool = ph3.enter_context(tc.tile_pool(name="oatu", bufs=17))
        den_pool = ph3.enter_context(tc.tile_pool(name="den", bufs=2))
        scr_pool = ph3.enter_context(tc.tile_pool(name="scr", bufs=4))
        Copy = mybir.ActivationFunctionType.Copy
        for ib in range(IBLK):
            den = den_pool.tile([H, NB], F32, tag="den", name=f"den{ib}")
            oatu = {}
            # scores -> exp -> PV (+denominator row) for all heads of this i-block
            for ot in range(OT):
                for hh in range(2):
                    h = 2 * ot + hh
                    lo, hi = hh * D, (hh + 1) * D
                    probs = []
                    for jt in range(JT):
                        ps = st_ps.tile([P, NB], F32, tag="st", name=f"st{h}_{ib}_{jt}")
                        nc.tensor.matmul(
                            out=ps,
                            lhsT=kt[ot][lo:hi, jt * P : (jt + 1) * P],
                            rhs=qt[ot][lo:hi, ib * NB : (ib + 1) * NB],
                            start=True,
                            stop=True,
                        )
                        pt = probs_pool.tile(
                            [P, NB], MDT, tag="probs", name=f"pr{h}_{ib}_{jt}"
                        )
                        nc.scalar.activation(pt, ps, Exp)
                        probs.append(pt)
                    pv = pv_ps.tile([65, NB], F32, tag="pv", name=f"pv{h}_{ib}")
                    for jt in range(JT):
                        nc.tensor.matmul(
                            out=pv,
                            lhsT=v_aug[jt][:, h, :],
                            rhs=probs[jt][:],
                            start=(jt == 0),
                            stop=(jt == JT - 1),
                        )
                    # evacuate unnormalized attn out (ACT) + denominator row (DMA)
                    ou = oatu_pool.tile([65, NB], F32, tag="oatu", name=f"ou{h}_{ib}")
                    nc.scalar.activation(ou, pv[:], Copy)
                    oatu[h] = ou
                    nc.sync.dma_start(out=den[h : h + 1, :], in_=ou[64:65, :])
            # one batched reciprocal for all 16 heads, bounced through DRAM
            rec = den_pool.tile([H, NB], F32, tag="rec", name=f"rec{ib}")
            nc.vector.reciprocal(rec, den)
            nc.sync.dma_start(out=recd[ib], in_=rec)
            # normalize: oat[h] = oatu[h] * bcast(rec[h])
            for ot in range(OT):
                for hh in range(2):
                    h = 2 * ot + hh
                    bcs = scr_pool.tile([D, NB], F32, tag="bcs", name=f"bs{h}_{ib}")
                    src = recd[ib, h, :]
                    nc.sync.dma_start(
                        out=bcs,
                        in_=bass.AP(
                            tensor=src.tensor, offset=src.offset, ap=[[0, D]] + list(src.ap)
                        ),
                    )
                    if hh == 0:
                        nc.vector.tensor_mul(
                            oat[ot][0:64, ib * NB : (ib + 1) * NB],
                            oatu[h][0:64, :],
                            bcs[:],
                        )
                    else:
                        sc = scr_pool.tile([D, NB], MDT, tag="scr", name=f"sc{h}_{ib}")
                        nc.vector.tensor_mul(sc, oatu[h][0:64, :], bcs[:])
                        nc.sync.dma_start(
                            out=oat[ot][64:128, ib * NB : (ib + 1) * NB], in_=sc
                        )

    # ================= phase 4: output projection: out[i, m] =================
    outsb_pool = ctx.enter_context(tc.tile_pool(name="outsb", bufs=3))
    for isub in range(NQ // P):
        for mb in range(MBLK):
            ps = proj_ps.tile([P, NB], F32, tag="proj", name=f"ops{isub}_{mb}")
            for ot in range(OT):
                nc.tensor.matmul(
                    out=ps,
                    lhsT=oat[ot][:, isub * P : (isub + 1) * P],
                    rhs=wo_t[ot][:, mb * NB : (mb + 1) * NB],
                    start=(ot == 0),
                    stop=(ot == OT - 1),
                )
            ob = outsb_pool.tile([P, NB], F32, tag="outsb", name=f"ob{isub}_{mb}")
            nc.vector.tensor_add(ob, ps, bo_sb[:, mb * NB : (mb + 1) * NB])
            nc.sync.dma_start(
                out=out[isub * P : (isub + 1) * P, mb * NB : (mb + 1) * NB], in_=ob
            )

    ctx.close()


def build_nc(mm_dt=mybir.dt.float32r):
    nc = bacc.Bacc(
        "TRN2",
        target_bir_lowering=False,
        debug=False,
        num_devices=N_CORES,
    )
    aps = (
        nc.dram_tensor("xT", [IN_DIM, NQ], mm_dt, kind="ExternalInput").ap(),
        nc.dram_tensor("ctxT", [CTX_DIM, NKV], mm_dt, kind="ExternalInput").ap(),
        nc.dram_tensor("wqT", [IN_DIM, IN_DIM], mm_dt, kind="ExternalInput").ap(),
        nc.dram_tensor("wkT", [CTX_DIM, IN_DIM], mm_dt, kind="ExternalInput").ap(),
        nc.dram_tensor("wvT", [CTX_DIM, IN_DIM], mm_dt, kind="ExternalInput").ap(),
        nc.dram_tensor("woT", [IN_DIM, IN_DIM], mm_dt, kind="ExternalInput").ap(),
        nc.dram_tensor("bq", [IN_DIM], F32, kind="ExternalInput").ap(),
        nc.dram_tensor("bk", [IN_DIM], F32, kind="ExternalInput").ap(),
        nc.dram_tensor("bv", [IN_DIM], F32, kind="ExternalInput").ap(),
        nc.dram_tensor("bo", [IN_DIM], F32, kind="ExternalInput").ap(),
        nc.dram_tensor("out", [NQ, IN_DIM], F32, kind="ExternalOutput").ap(),
    )
    with tile.TileContext(nc) as tc:
        _emit(tc, aps, mm_dt)
    nc.compile()
    return nc


_NC_CACHE = {}


def get_nc(mm_dt=mybir.dt.float32r):
    key = str(mm_dt)
    if key not in _NC_CACHE:
        _NC_CACHE[key] = build_nc(mm_dt)
    return _NC_CACHE[key]


DEFAULT_MM_DT = mybir.dt.float32r


def make_in_maps(x, context, Wq, bq, Wk, bk, Wv, bv, Wo, bo, mm_dt=DEFAULT_MM_DT):
    f = lambda a: np.asarray(a, dtype=np.float32)
    md = mybir.dt.np(mm_dt)
    g = lambda a: np.ascontiguousarray(a).astype(md, copy=False)
    WqT = g(f(Wq).T * SCALE)
    WkT = g(f(Wk).T)
    WvT = g(f(Wv).T)
    WoT = g(f(Wo).T)
    bq_s = f(bq) * SCALE
    bk, bv, bo = f(bk), f(bv), f(bo)
    x, context = f(x), f(context)
    in_maps = []
    for b in range(B):
        in_maps.append(
            {
                "xT": g(x[b].T),
                "ctxT": g(context[b].T),
                "wqT": WqT,
                "wkT": WkT,
                "wvT": WvT,
                "woT": WoT,
                "bq": bq_s,
                "bk": bk,
                "bv": bv,
                "bo": bo,
            }
        )
    return in_maps


def run(in_maps, mm_dt=DEFAULT_MM_DT, trace=False, **kw):
    nc = get_nc(mm_dt)
    return run_bass_kernel_spmd(nc, in_maps, list(range(N_CORES)), trace=trace, **kw)


def kernel(x, context, Wq, bq, Wk, bk, Wv, bv, Wo, bo):
    in_maps = make_in_maps(x, context, Wq, bq, Wk, bk, Wv, bv, Wo, bo, DEFAULT_MM_DT)
    res = run(in_maps, DEFAULT_MM_DT).results
    return np.stack([res[b]["out"] for b in range(B)], axis=0)
